# revision 12
# baseline (speedup 1.0000x reference)
"""DiT block kernel for 8 Trainium2 NeuronCores.

Sharding: core = (b, half) with b = core//2 in [0,4), half = core%2.
Each core computes output rows [half*1024:(half+1)*1024) of batch b.
K/V are computed for the full 2048 rows of the batch on both cores of a
pair (duplicated compute, no collectives needed).

Host-side prep folds the adaLN modulation into per-core weight/vector
inputs:
  mod = silu(c) @ W_ada + b_ada -> scale_msa, gate_msa, scale_mlp, gate_mlp
  g1' = g1*(1+scale_msa[b]),  b1' = b1*(1+scale_msa[b])
  g2' = g2*(1+scale_mlp[b]),  b2' = b2*(1+scale_mlp[b])
  Wo' = Wo * gate_msa[b][None,:]
  W2' = W_mlp2 * gate_mlp[b][None,:],  b2m' = b_mlp2*gate_mlp[b]
Weights are pre-cast to bf16 on the host (matmul operand precision; all
accumulation on device is fp32, residual path stays fp32).

Attention uses the transposed-scores orientation: sT[tk,tq] = kT.T @ qT
(both operands produced transposed by the QKV matmuls), exp on ACT from
PSUM, then oT[d,tq] = [v|1].T @ expST which yields the per-query softmax
denominator in partition 64 of the PSUM output.
"""

import numpy as np
import ml_dtypes

P = 128
D = 512
T = 2048
TL = 1024  # rows per core
H = 8
DK = 64
HID = 2048
EPS = 1e-5
NCORES = 8

BF16 = ml_dtypes.bfloat16

_cache = {}


def _split_excess_waits(nc):
    """This walrus build accepts at most 1 sync wait per instruction (2 for
    EVENT_SEMAPHORE), but Tile can attach more. Move excess waits onto
    ENGINE_NOP carriers inserted just before the instruction on the same
    engine (sequencer program order preserves the wait semantics)."""
    from concourse import mybir

    Op = nc.isa.Opcode

    def is_evsem(inst):
        return (
            isinstance(inst, mybir.InstISA)
            and inst.isa_opcode == Op.NEURON_ISA_TPB_OPCODE_EVENT_SEMAPHORE.value
        ) or "EventSem" in type(inst).__name__

    for f in nc.m.functions:
        for bb in f.blocks:
            out = []
            changed = False
            for inst in bb.instructions:
                si = inst.sync_info
                keep = 2 if is_evsem(inst) else 1
                if si is not None and len(si.on_wait) > keep:
                    excess = list(si.on_wait[:-keep])
                    del si.on_wait[:-keep]
                    for w in excess:
                        n = nc.engines[inst.engine]._isa(
                            Op.NEURON_ISA_TPB_OPCODE_NOP, {}
                        )
                        n.sync_info = mybir.SyncInfo(on_wait=[w], on_update=[])
                        out.append(n)
                    changed = True
                out.append(inst)
            if changed:
                bb.instructions[:] = out


def _bcast_ap(bass, dram_ap, parts):
    """[n] DRAM AP -> [parts, n] partition-broadcast AP."""
    return bass.AP(
        tensor=dram_ap.tensor,
        offset=dram_ap.offset,
        ap=[[0, parts], *dram_ap.ap],
    )


def _build_program(has_b1, has_b2, has_bm1, has_bm2):
    import concourse.bass as bass
    import concourse.tile as tile
    from concourse import mybir
    from concourse.masks import make_identity
    from contextlib import ExitStack

    f32 = mybir.dt.float32
    bf16 = mybir.dt.bfloat16
    AF = mybir.ActivationFunctionType
    OP = mybir.AluOpType

    nc = bass.Bass("TRN2", target_bir_lowering=False, debug=False)

    # ---- I/O ----
    xm = nc.dram_tensor("xm", [TL, D], f32, kind="ExternalInput").ap()
    xo = nc.dram_tensor("xo", [TL, D], f32, kind="ExternalInput").ap()
    wq_d = nc.dram_tensor("wq", [D, D], bf16, kind="ExternalInput").ap()
    wk_d = nc.dram_tensor("wk", [D, D], bf16, kind="ExternalInput").ap()
    wv_d = nc.dram_tensor("wv", [D, D], bf16, kind="ExternalInput").ap()
    wo_d = nc.dram_tensor("wo", [D, D], bf16, kind="ExternalInput").ap()
    w1_d = nc.dram_tensor("w1", [D, HID], bf16, kind="ExternalInput").ap()
    w2_d = nc.dram_tensor("w2", [HID, D], bf16, kind="ExternalInput").ap()
    g1_d = nc.dram_tensor("g1", [D], f32, kind="ExternalInput").ap()
    g2_d = nc.dram_tensor("g2", [D], f32, kind="ExternalInput").ap()
    b1_d = nc.dram_tensor("b1", [D], f32, kind="ExternalInput").ap() if has_b1 else None
    b2_d = nc.dram_tensor("b2", [D], f32, kind="ExternalInput").ap() if has_b2 else None
    bm1_d = (
        nc.dram_tensor("bm1", [HID], f32, kind="ExternalInput").ap() if has_bm1 else None
    )
    bm2_d = (
        nc.dram_tensor("bm2", [D], f32, kind="ExternalInput").ap() if has_bm2 else None
    )
    y = nc.dram_tensor("y", [TL, D], f32, kind="ExternalOutput").ap()

    xm_t = xm.rearrange("(n p) d -> n p d", p=P)  # [8, 128, 512]
    xo_t = xo.rearrange("(n p) d -> n p d", p=P)
    y_t = y.rearrange("(n p) d -> n p d", p=P)
    wq_r = wq_d.rearrange("(o p) n -> p o n", p=P)  # [128, 4, 512]
    wk_r = wk_d.rearrange("(o p) n -> p o n", p=P)
    wv_r = wv_d.rearrange("(o p) n -> p o n", p=P)
    wo_r = wo_d.rearrange("(o p) n -> p o n", p=P)
    w1_r = w1_d.rearrange("(o p) n -> p o n", p=P)  # [128, 4, 2048]
    w2_r = w2_d.rearrange("(o p) n -> p o n", p=P)  # [128, 16, 512]

    NT = T // P  # 16 row tiles for LN1
    NTL = TL // P  # 8 row tiles (mine)
    KS = D // P  # 4 contraction subtiles for D
    HC = HID // P  # 16 hidden chunks

    with tile.TileContext(nc) as tc, ExitStack() as ctx:
        singles = ctx.enter_context(tc.tile_pool(name="singles", bufs=1))
        big = ctx.enter_context(tc.tile_pool(name="big", bufs=2))
        scratch = ctx.enter_context(tc.tile_pool(name="scratch", bufs=3))
        xnbp = ctx.enter_context(tc.tile_pool(name="xnb", bufs=3))
        stats = ctx.enter_context(tc.tile_pool(name="stats", bufs=6))
        expSp = ctx.enter_context(tc.tile_pool(name="expS", bufs=3))
        denomp = ctx.enter_context(tc.tile_pool(name="denom", bufs=2))
        oddp = ctx.enter_context(tc.tile_pool(name="oddtmp", bufs=2))
        dramp = ctx.enter_context(tc.tile_pool(name="dram", bufs=4, space="DRAM"))

        # ---- resident tiles ----
        ident = singles.tile([P, P], bf16)
        make_identity(nc, ident)
        eps_t = singles.tile([P, 1], f32)
        nc.vector.memset(eps_t, EPS)

        g1B = singles.tile([P, D], f32)
        nc.gpsimd.dma_start(out=g1B, in_=_bcast_ap(bass, g1_d, P))
        g2B = singles.tile([P, D], f32)
        nc.gpsimd.dma_start(out=g2B, in_=_bcast_ap(bass, g2_d, P))
        b1B = b2B = bm2B = None
        if has_b1:
            b1B = singles.tile([P, D], f32)
            nc.gpsimd.dma_start(out=b1B, in_=_bcast_ap(bass, b1_d, P))
        if has_b2:
            b2B = singles.tile([P, D], f32)
            nc.gpsimd.dma_start(out=b2B, in_=_bcast_ap(bass, b2_d, P))
        if has_bm2:
            bm2B = singles.tile([P, D], f32)
            nc.gpsimd.dma_start(out=bm2B, in_=_bcast_ap(bass, bm2_d, P))
        bm1_sb = None
        if has_bm1:
            bm1_sb = singles.tile([P, HC], f32)
            nc.sync.dma_start(out=bm1_sb, in_=bm1_d.rearrange("(o p) -> p o", p=P))

        wq_sb = singles.tile([P, KS, D], bf16)
        nc.sync.dma_start(out=wq_sb, in_=wq_r)
        wk_sb = singles.tile([P, KS, D], bf16)
        nc.sync.dma_start(out=wk_sb, in_=wk_r)
        wv_sb = singles.tile([P, KS, D], bf16)
        nc.sync.dma_start(out=wv_sb, in_=wv_r)
        wo_sb = singles.tile([P, KS, D], bf16)
        nc.sync.dma_start(out=wo_sb, in_=wo_r)
        w2_sb = singles.tile([P, HC, D], bf16)
        nc.sync.dma_start(out=w2_sb, in_=w2_r)

        x_res = singles.tile([P, NTL, D], f32)  # my rows; becomes x1, then y
        qT = singles.tile([P, KS, TL], bf16)  # [dmix, chunk, tq] (pre-scaled 1/8)
        v_sb = singles.tile([P, NT, H, DK + 1], bf16)  # ones column at [.., 64]
        nc.gpsimd.memset(v_sb, 1.0)
        oT = singles.tile([P, KS, TL], bf16)
        xn2T = singles.tile([P, KS, TL], bf16)
        hT = singles.tile([P, HC, TL // 2], bf16)  # one tq-half at a time

        # big pool rotation: xn1T -> (kT lives in parallel) -> w1 reuses slot
        xn1T = big.tile([P, KS, T], bf16, tag="big")
        kT = big.tile([P, KS, T], bf16, tag="big")
        # w1 allocated later from same tag

        # ================= Phase 1: LN1 + transpose =================
        with tc.tile_pool(name="xps", bufs=4, space="PSUM") as xps:
            for i in range(NT):
                if i < NTL:
                    xt_ap = x_res[:, i, :]
                    nc.sync.dma_start(out=xt_ap, in_=xm_t[i])
                else:
                    xot = scratch.tile([P, D], f32, tag="xo")
                    nc.sync.dma_start(out=xot, in_=xo_t[i - NTL])
                    xt_ap = xot[:]
                st = stats.tile([P, 6], f32, tag="st")
                nc.vector.bn_stats(out=st, in_=xt_ap)
                mv = stats.tile([P, 2], f32, tag="mv")
                nc.vector.bn_aggr(out=mv, in_=st)
                rstd = stats.tile([P, 1], f32, tag="rstd")
                nc.scalar.activation(rstd, mv[:, 1:2], AF.Sqrt, bias=eps_t)
                nc.vector.reciprocal(rstd, rstd)
                xc = scratch.tile([P, D], f32, tag="xc")
                nc.vector.tensor_scalar(
                    out=xc,
                    in0=xt_ap,
                    scalar1=mv[:, 0:1],
                    scalar2=rstd,
                    op0=OP.subtract,
                    op1=OP.mult,
                )
                xnb = xnbp.tile([P, D], bf16, tag="xnb")
                nc.vector.scalar_tensor_tensor(
                    out=xnb, in0=xc, scalar=0.0, op0=OP.bypass, in1=g1B, op1=OP.mult
                )
                if has_b1:
                    nc.vector.tensor_add(out=xnb, in0=xnb, in1=b1B)
                for c in range(KS):
                    tp = xps.tile([P, P], bf16, tag="tp")
                    nc.tensor.transpose(tp, xnb[:, c * P : (c + 1) * P], ident)
                    nc.scalar.copy(out=xn1T[:, c, i * P : (i + 1) * P], in_=tp)

            # ================= Phase 2: QKV =================
            with tc.tile_pool(name="qkvps", bufs=4, space="PSUM") as qkvps:
                NB = D // D  # helper no-op
                # qT and kT (transposed outputs)
                for c in range(KS):
                    for nb in range(TL // 512):  # qT: my rows only
                        ps = qkvps.tile([P, 512], f32, tag="ps")
                        for ks in range(KS):
                            nc.tensor.matmul(
                                ps,
                                lhsT=wq_sb[:, ks, c * P : (c + 1) * P],
                                rhs=xn1T[:, ks, nb * 512 : (nb + 1) * 512],
                                start=(ks == 0),
                                stop=(ks == KS - 1),
                            )
                        nc.scalar.mul(
                            out=qT[:, c, nb * 512 : (nb + 1) * 512], in_=ps,
                            mul=1.0 / np.sqrt(DK),
                        )
                    for nb in range(T // 512):  # kT: all rows
                        ps = qkvps.tile([P, 512], f32, tag="ps")
                        for ks in range(KS):
                            nc.tensor.matmul(
                                ps,
                                lhsT=wk_sb[:, ks, c * P : (c + 1) * P],
                                rhs=xn1T[:, ks, nb * 512 : (nb + 1) * 512],
                                start=(ks == 0),
                                stop=(ks == KS - 1),
                            )
                        nc.vector.tensor_copy(
                            out=kT[:, c, nb * 512 : (nb + 1) * 512], in_=ps
                        )
                # v in natural layout (+ ones column preserved)
                for t in range(NT):
                    ps = qkvps.tile([P, 512], f32, tag="ps")
                    for ks in range(KS):
                        nc.tensor.matmul(
                            ps,
                            lhsT=xn1T[:, ks, t * P : (t + 1) * P],
                            rhs=wv_sb[:, ks, :],
                            start=(ks == 0),
                            stop=(ks == KS - 1),
                        )
                    nc.vector.tensor_copy(
                        out=v_sb[:, t, :, 0:DK],
                        in_=ps.rearrange("p (h d) -> p h d", d=DK),
                    )

        # ================= Phase 3+4: attention + Wo + residual =================
        with (
            tc.tile_pool(name="scA", bufs=1, space="PSUM") as scA,
            tc.tile_pool(name="scB", bufs=1, space="PSUM") as scB,
            tc.tile_pool(name="pvps", bufs=2, space="PSUM") as pvp,
            tc.tile_pool(name="wops", bufs=2, space="PSUM") as wop,
        ):
            NQB = TL // 512  # 2 query blocks
            NTK = T // P  # 16 key subtiles
            for qb in range(NQB):
                qsl = slice(qb * 512, (qb + 1) * 512)
                for hp in range(H // 2):
                    pva = pvp.tile([DK + 1, 512], f32, tag="pv")
                    pvb = pvp.tile([DK + 1, 512], f32, tag="pv")
                    for g in range(NTK // 2):
                        sa = scA.tile([P, 2, 512], f32, tag="sa")
                        sb_ = scB.tile([P, 2, 512], f32, tag="sb")
                        for j in range(2):
                            tk = 2 * g + j
                            tksl = slice(tk * P, (tk + 1) * P)
                            nc.tensor.matmul(
                                sa[:, j, :],
                                lhsT=kT[0:DK, hp, tksl],
                                rhs=qT[0:DK, hp, qsl],
                                start=True,
                                stop=True,
                            )
                            nc.tensor.matmul(
                                sb_[:, j, :],
                                lhsT=kT[DK:P, hp, tksl],
                                rhs=qT[DK:P, hp, qsl],
                                start=True,
                                stop=True,
                            )
                        ea = expSp.tile([P, 2, 512], bf16, tag="ea")
                        eb = expSp.tile([P, 2, 512], bf16, tag="eb")
                        nc.scalar.activation(out=ea, in_=sa, func=AF.Exp)
                        nc.scalar.activation(out=eb, in_=sb_, func=AF.Exp)
                        for j in range(2):
                            tk = 2 * g + j
                            nc.tensor.matmul(
                                pva,
                                lhsT=v_sb[:, tk, 2 * hp, 0 : DK + 1],
                                rhs=ea[:, j, :],
                                start=(tk == 0),
                                stop=(tk == NTK - 1),
                                skip_group_check=True,
                            )
                            nc.tensor.matmul(
                                pvb,
                                lhsT=v_sb[:, tk, 2 * hp + 1, 0 : DK + 1],
                                rhs=eb[:, j, :],
                                start=(tk == 0),
                                stop=(tk == NTK - 1),
                                skip_group_check=True,
                            )
                    # normalize by the summed ones column (partition DK) and
                    # place oT[d, tq]: even head -> partitions 0:64 of chunk,
                    # odd head -> partitions 64:128 (via DMA partition shift).
                    for h_i, pv in ((0, pva), (1, pvb)):
                        rb = denomp.tile([P, 512], f32, tag="rb")
                        nc.vector.reciprocal(rb[DK : DK + 1, :], pv[DK : DK + 1, :])
                        dr = dramp.tile([1, 512], f32, tag="dr")
                        nc.sync.dma_start(out=dr, in_=rb[DK : DK + 1, :])
                        nc.gpsimd.dma_start(
                            out=rb[0:DK, :],
                            in_=dr[:, :].to_broadcast([DK, 512]),
                        )
                        if h_i == 0:
                            nc.vector.tensor_mul(
                                out=oT[0:DK, hp, qsl], in0=pv[0:DK, :], in1=rb[0:DK, :]
                            )
                        else:
                            ot = oddp.tile([DK, 512], bf16, tag="ot")
                            nc.vector.tensor_mul(
                                out=ot, in0=pv[0:DK, :], in1=rb[0:DK, :]
                            )
                            nc.gpsimd.dma_start(out=oT[DK:P, hp, qsl], in_=ot)
                # Wo + residual for this query block
                for tt in range(qb * 4, qb * 4 + 4):
                    ps = wop.tile([P, D], f32, tag="wo")
                    for ks in range(KS):
                        nc.tensor.matmul(
                            ps,
                            lhsT=oT[:, ks, tt * P : (tt + 1) * P],
                            rhs=wo_sb[:, ks, :],
                            start=(ks == 0),
                            stop=(ks == KS - 1),
                        )
                    nc.vector.tensor_add(
                        out=x_res[:, tt, :], in0=x_res[:, tt, :], in1=ps
                    )

        # ================= Phase 5: LN2 + transpose =================
        with tc.tile_pool(name="xps2", bufs=4, space="PSUM") as xps2:
            for tt in range(NTL):
                xt_ap = x_res[:, tt, :]
                st = stats.tile([P, 6], f32, tag="st")
                nc.vector.bn_stats(out=st, in_=xt_ap)
                mv = stats.tile([P, 2], f32, tag="mv")
                nc.vector.bn_aggr(out=mv, in_=st)
                rstd = stats.tile([P, 1], f32, tag="rstd")
                nc.scalar.activation(rstd, mv[:, 1:2], AF.Sqrt, bias=eps_t)
                nc.vector.reciprocal(rstd, rstd)
                xc = scratch.tile([P, D], f32, tag="xc")
                nc.vector.tensor_scalar(
                    out=xc,
                    in0=xt_ap,
                    scalar1=mv[:, 0:1],
                    scalar2=rstd,
                    op0=OP.subtract,
                    op1=OP.mult,
                )
                xnb = xnbp.tile([P, D], bf16, tag="xnb")
                nc.vector.scalar_tensor_tensor(
                    out=xnb, in0=xc, scalar=0.0, op0=OP.bypass, in1=g2B, op1=OP.mult
                )
                if has_b2:
                    nc.vector.tensor_add(out=xnb, in0=xnb, in1=b2B)
                for c in range(KS):
                    tp = xps2.tile([P, P], bf16, tag="tp")
                    nc.tensor.transpose(tp, xnb[:, c * P : (c + 1) * P], ident)
                    nc.scalar.copy(out=xn2T[:, c, tt * P : (tt + 1) * P], in_=tp)

        # ================= Phase 6: MLP =================
        w1_sb = big.tile([P, KS, HID], bf16, tag="big")
        nc.sync.dma_start(out=w1_sb, in_=w1_r)
        with (
            tc.tile_pool(name="hps", bufs=2, space="PSUM") as hps,
            tc.tile_pool(name="o2ps", bufs=2, space="PSUM") as o2ps,
        ):
            for half in range(2):
                hsl = slice(half * 512, (half + 1) * 512)
                for hcg in range(HC // 2):
                    hp_t = hps.tile([P, 2, 512], f32, tag="h")
                    for j in range(2):
                        hc = 2 * hcg + j
                        for ks in range(KS):
                            nc.tensor.matmul(
                                hp_t[:, j, :],
                                lhsT=w1_sb[:, ks, hc * P : (hc + 1) * P],
                                rhs=xn2T[:, ks, hsl],
                                start=(ks == 0),
                                stop=(ks == KS - 1),
                            )
                    if has_bm1:
                        for j in range(2):
                            hc = 2 * hcg + j
                            nc.scalar.activation(
                                out=hT[:, hc, :],
                                in_=hp_t[:, j, :],
                                func=AF.Gelu,
                                bias=bm1_sb[:, hc : hc + 1],
                            )
                    else:
                        nc.scalar.activation(
                            out=hT[:, 2 * hcg : 2 * hcg + 2, :], in_=hp_t, func=AF.Gelu
                        )
                for tl in range(4):
                    tt = half * 4 + tl
                    o2 = o2ps.tile([P, D], f32, tag="o2")
                    for hc in range(HC):
                        nc.tensor.matmul(
                            o2,
                            lhsT=hT[:, hc, tl * P : (tl + 1) * P],
                            rhs=w2_sb[:, hc, :],
                            start=(hc == 0),
                            stop=(hc == HC - 1),
                        )
                    ot = scratch.tile([P, D], f32, tag="out")
                    if has_bm2:
                        nc.vector.tensor_add(out=ot, in0=o2, in1=bm2B)
                        nc.vector.tensor_add(out=ot, in0=ot, in1=x_res[:, tt, :])
                    else:
                        nc.vector.tensor_add(out=ot, in0=x_res[:, tt, :], in1=o2)
                    nc.sync.dma_start(out=y_t[tt], in_=ot)

    _split_excess_waits(nc)
    return nc


def _host_prep(inputs):
    x = np.asarray(inputs["x"], np.float32)
    c = np.asarray(inputs["c"], np.float32)
    mod = (c / (1.0 + np.exp(-c))) @ np.asarray(inputs["W_ada"], np.float32)
    mod = mod + np.asarray(inputs["b_ada"], np.float32)
    scale_msa, gate_msa, scale_mlp, gate_mlp = np.split(mod, 4, axis=-1)

    g1 = np.asarray(inputs["g1"], np.float32)
    b1 = np.asarray(inputs["b1"], np.float32)
    g2 = np.asarray(inputs["g2"], np.float32)
    b2 = np.asarray(inputs["b2"], np.float32)
    Wo = np.asarray(inputs["Wo"], np.float32)
    W2 = np.asarray(inputs["W_mlp2"], np.float32)
    bm2 = np.asarray(inputs["b_mlp2"], np.float32)

    per_core = []
    for core in range(NCORES):
        b = core // 2
        half = core % 2
        s1 = 1.0 + scale_msa[b]
        s2 = 1.0 + scale_mlp[b]
        m = {
            "xm": np.ascontiguousarray(x[b, half * TL : (half + 1) * TL]),
            "xo": np.ascontiguousarray(x[b, (1 - half) * TL : (2 - half) * TL]),
            "wq": np.asarray(inputs["Wq"], np.float32).astype(BF16),
            "wk": np.asarray(inputs["Wk"], np.float32).astype(BF16),
            "wv": np.asarray(inputs["Wv"], np.float32).astype(BF16),
            "wo": (Wo * gate_msa[b][None, :]).astype(BF16),
            "w1": np.asarray(inputs["W_mlp1"], np.float32).astype(BF16),
            "w2": (W2 * gate_mlp[b][None, :]).astype(BF16),
            "g1": (g1 * s1).astype(np.float32),
            "g2": (g2 * s2).astype(np.float32),
        }
        _b1 = (b1 * s1).astype(np.float32)
        _b2 = (b2 * s2).astype(np.float32)
        _bm1 = np.asarray(inputs["b_mlp1"], np.float32)
        _bm2 = (bm2 * gate_mlp[b]).astype(np.float32)
        m["_flags"] = (
            bool(np.any(_b1)), bool(np.any(_b2)),
            bool(np.any(_bm1)), bool(np.any(_bm2)),
        )
        if m["_flags"][0]:
            m["b1"] = _b1
        if m["_flags"][1]:
            m["b2"] = _b2
        if m["_flags"][2]:
            m["bm1"] = _bm1
        if m["_flags"][3]:
            m["bm2"] = _bm2
        per_core.append(m)
    return per_core


def kernel(**inputs):
    from concourse import bass_utils

    per_core = _host_prep(inputs)
    flags = per_core[0]["_flags"]
    for m in per_core:
        assert m["_flags"] == flags
        del m["_flags"]

    if ("nc", flags) not in _cache:
        _cache[("nc", flags)] = _build_program(*flags)
    nc = _cache[("nc", flags)]

    res = bass_utils.run_bass_kernel_spmd(nc, per_core, core_ids=list(range(NCORES)))

    x = inputs["x"]
    out = np.empty((x.shape[0], T, D), np.float32)
    for core in range(NCORES):
        b = core // 2
        half = core % 2
        out[b, half * TL : (half + 1) * TL] = res.results[core]["y"]
    return out
